# revision 1
# baseline (speedup 1.0000x reference)
"""Multi-head latent attention (MLA) prefill kernel for 8 Trainium2 NeuronCores.

Sharding strategy (token-parallel projections + head-parallel attention):
  Phase A (token-parallel): each core computes, for ITS 512 of the 4096
    tokens, the compressed latents c_q/c_kv AND the full q_c/q_r/k_r
    up-projections for ALL 16 heads (same FLOPs as projecting 2 heads for
    all tokens, but no c_q AllGather and no re-load of x). c_kv is
    AllGathered (4.2MB) for the k/v up-projections; q_c and q_r/k_r are
    exchanged with two 2.1MB AllToAlls that land each core its 2 heads for
    all 4096 tokens.
  Phase B (head-parallel): each core up-projects k_c and v^T for its 2
    heads from the gathered c_kv, then runs causal attention.
  Phase C: an AllToAll token-shards the attention output O so each core
    applies the full out-projection to its 512 tokens.

Attention inner loop works on PAIRS of 128-key blocks: scores for pair j+1
are issued before the ov matmuls of pair j (tensor engine never stalls on
the exp), exp runs once per pair on a 2-bank [128,1024] PSUM tile, and the
softmax denominator accumulates on the vector engine (scores are ~N(0,1)
after scaling so exp cannot overflow); a single f32r ones-matmul per
q-block broadcasts the denominator across partitions. v^T is computed
directly (stationary = c_kv token block, moving = w_uv^T both heads)
instead of v followed by PE transposes.

Weight streams are host-packed so each tile load is a single DMA with long
contiguous runs. Out-projection head-0 partials stay resident in SBUF.
"""

import sys
import types

sys.path.insert(0, "/opt/trn_rl_repo")

import ml_dtypes
import numpy as np

from concourse import bacc, bass, mybir, tile
from concourse import bass_utils

F32 = mybir.dt.float32
F32R = mybir.dt.float32r
BF16 = mybir.dt.bfloat16
AF = mybir.ActivationFunctionType

E = 2048
H = 16
HD = 128
CKV = 512
CQ = 1536
RD = 64
SCALE = 1.0 / np.sqrt(HD + RD)
B = 2
S = 2048
T = B * S            # 4096 tokens
NC = 8               # cores
TPC = T // NC        # 512 tokens per core
HPC = H // NC        # 2 heads per core
NB = T // 512        # 8 token blocks of 512
NBB = S // 512       # 4 token blocks per batch
ET = E // 128        # 16 e-tiles
CQT = CQ // 128      # 12 c_q tiles
CKVT = CKV // 128    # 4 c_kv tiles
QCC = H              # 16 q_c output chunks of 128 dims
QRC = H * RD // 128  # 8 q_r / k_r output chunks of 128 dims


def build_program():
    nc = bacc.Bacc("TRN2", target_bir_lowering=False, debug=False, num_devices=NC)

    # ---- I/O ----
    # *_p tensors are host-packed to [128 partitions, ...] so tile loads are
    # single DMAs with long contiguous runs.
    x_p = nc.dram_tensor("x_p", [128, ET * TPC], BF16, kind="ExternalInput")
    wdq_p = nc.dram_tensor("wdq_p", [128, CQT * ET * 128], BF16, kind="ExternalInput")
    wdkv_p = nc.dram_tensor("wdkv_p", [128, CKVT * ET * 128], BF16, kind="ExternalInput")
    wkr_p = nc.dram_tensor("wkr_p", [128, QRC * ET * 128], BF16, kind="ExternalInput")
    wuq_p = nc.dram_tensor("wuq_p", [128, QCC * CQT * 128], BF16, kind="ExternalInput")
    wqr_p = nc.dram_tensor("wqr_p", [128, QRC * CQT * 128], BF16, kind="ExternalInput")
    wuk_p = nc.dram_tensor("wuk_p", [128, CKVT * 256], BF16, kind="ExternalInput")
    wuv_p = nc.dram_tensor("wuv_p", [128, CKVT * 256], BF16, kind="ExternalInput")
    wout_p = nc.dram_tensor("wout_p", [128, ET * ET * 128], BF16, kind="ExternalInput")
    cos_t = nc.dram_tensor("cos_t", [128, 512], F32, kind="ExternalInput")
    sin_t = nc.dram_tensor("sin_t", [128, 512], F32, kind="ExternalInput")
    mask_t = nc.dram_tensor("mask_t", [128, 4 * 512], BF16, kind="ExternalInput")
    ones_f32_t = nc.dram_tensor("ones_f32_t", [128, 128], F32R, kind="ExternalInput")
    out_t = nc.dram_tensor("out_t", [E, TPC], F32, kind="ExternalOutput")

    # ---- internal DRAM (collective bounce buffers) ----
    ag_in0 = nc.dram_tensor("ag_in0", [CKV, TPC], BF16)
    ag_out0 = nc.dram_tensor("ag_out0", [NC * CKV, TPC], BF16, addr_space="Shared")
    # shard j of qc: q_c rows for heads 2j/2j+1 (my tokens);
    # shard j of qrkr: q_r rows (128) then k_r rows (128) for heads 2j/2j+1.
    qc_a2a_in = nc.dram_tensor("qc_a2a_in", [NC * 256, 512], BF16)
    qc_a2a_out = nc.dram_tensor("qc_a2a_out", [NC * 256, 512], BF16)
    qrkr_a2a_in = nc.dram_tensor("qrkr_a2a_in", [NC * 256, 512], BF16)
    qrkr_a2a_out = nc.dram_tensor("qrkr_a2a_out", [NC * 256, 512], BF16)
    a2a_in = [nc.dram_tensor(f"a2a_in{h}", [NC * HD, 512], BF16) for h in range(HPC)]
    a2a_out = [nc.dram_tensor(f"a2a_out{h}", [NC * HD, 512], BF16) for h in range(HPC)]

    rg = [list(range(NC))]

    with tile.TileContext(nc) as tc:
        with (
            tc.tile_pool(name="pc_const", bufs=1) as pc_const,
            tc.tile_pool(name="pc_small", bufs=2) as pc_small,
        ):
            # constants resident for the whole kernel
            mask_sb = pc_const.tile([128, 4 * 512], BF16)
            nc.sync.dma_start(mask_sb[:], mask_t[:, :])
            wuk_sb = pc_const.tile([128, CKVT * 256], BF16)
            nc.sync.dma_start(wuk_sb[:], wuk_p[:, :])
            # wuv packed transposed: wuvT_sb[p, c*256+hh] = w_uv[hp hh, c*128+p]
            wuvT_sb = pc_const.tile([128, CKVT * 256], BF16)
            nc.sync.dma_start(wuvT_sb[:], wuv_p[:, :])
            ones_r = pc_const.tile([128, 128], F32R)
            nc.sync.dma_start(ones_r[:], ones_f32_t[:, :])
            cos_sb = pc_const.tile([128, 512], F32)
            nc.sync.dma_start(cos_sb[:], cos_t[:, :])
            sin_sb = pc_const.tile([128, 512], F32)
            nc.sync.dma_start(sin_sb[:], sin_t[:, :])

            def rope_local(dst, src_ps):
                """dst[:, 512] = rope(src_ps[:, 512]) for my token block.

                Rows are 64-dim RoPE blocks (one per head); rotate-half pairs
                row d with d+32 inside each block. sin comes pre-signed.
                """
                sh = pc_small.tile([128, 512], F32, tag="sh", bufs=2, name="sh")
                for blk in range(2):
                    p0 = blk * 64
                    nc.vector.tensor_copy(sh[p0 : p0 + 32, :], src_ps[p0 + 32 : p0 + 64, :])
                    nc.vector.tensor_copy(sh[p0 + 32 : p0 + 64, :], src_ps[p0 : p0 + 32, :])
                t1 = pc_small.tile([128, 512], F32, tag="t1", bufs=2, name="t1")
                nc.vector.tensor_mul(t1[:], src_ps[:], cos_sb[:])
                nc.vector.tensor_mul(sh[:], sh[:], sin_sb[:])
                nc.vector.tensor_add(dst, t1[:], sh[:])

            # ============ Phase A: token-local projections (my 512 tokens) ==
            with (
                tc.tile_pool(name="pa_x", bufs=1) as pa_x,
                tc.tile_pool(name="pa_we", bufs=12) as pa_we,
                tc.tile_pool(name="pa_wc", bufs=10) as pa_wc,
                tc.tile_pool(name="pa_s", bufs=3) as pa_s,
                tc.tile_pool(name="pa_loc", bufs=1) as pa_loc,
                tc.tile_pool(name="pa_ps", bufs=2, space="PSUM") as pa_ps,
            ):
                # first weight chunk is issued before the x stream so the
                # first chain's LDWEIGHTS isn't stuck behind 2MB of x
                w0_sb = pa_we.tile([128, ET * 128], BF16, tag="we", bufs=12, name="w_pre")
                nc.sync.dma_start(w0_sb[:], wdkv_p[:, : ET * 128])
                x_half = []
                for xh in range(2):
                    xt_ = pa_x.tile([128, 8 * TPC], BF16, tag=f"x{xh}", bufs=1, name=f"x{xh}")
                    # quarter-chunked so the first chain starts ~1 chunk in
                    for qtr in range(2):
                        nc.sync.dma_start(
                            xt_[:, qtr * 4 * TPC : (qtr + 1) * 4 * TPC],
                            x_p[:, (xh * 8 + qtr * 4) * TPC : (xh * 8 + (qtr + 1) * 4) * TPC],
                        )
                    x_half.append(xt_)
                cq_loc = pa_loc.tile([128, CQT * 512], BF16, tag="cq", bufs=1, name="cq_loc")

                def x_mov(e):
                    return x_half[e // 8][:, (e % 8) * TPC : (e % 8 + 1) * TPC]

                def cq_mov(c):
                    return cq_loc[:, c * 512 : (c + 1) * 512]

                def chain(idx, w_dram, w_off, w_cols, nmm, mov, rope, sink, sbuf_dst=None, w_pre=None):
                    """one output chunk: stream weight tile, accumulate, emit"""
                    if w_pre is not None:
                        w_sb = w_pre
                    else:
                        wtag, wpool, wb = ("we", pa_we, 12) if w_cols == ET * 128 else ("wc", pa_wc, 10)
                        w_sb = wpool.tile([128, w_cols], BF16, tag=wtag, bufs=wb, name=f"w{idx}")
                        nc.sync.dma_start(w_sb[:], w_dram[:, w_off : w_off + w_cols])
                    ps = pa_ps.tile([128, 512], F32, tag="pa", bufs=2, name=f"ps{idx}")
                    for k in range(nmm):
                        nc.tensor.matmul(
                            ps[:],
                            w_sb[:, k * 128 : (k + 1) * 128],
                            mov(k),
                            start=(k == 0),
                            stop=(k == nmm - 1),
                        )
                    if sbuf_dst is not None:
                        nc.vector.tensor_copy(sbuf_dst, ps[:])
                        return
                    o_sb = pa_s.tile([128, 512], BF16, tag="oa", bufs=3, name=f"oa{idx}")
                    if rope:
                        rope_local(o_sb[:], ps)
                    else:
                        nc.vector.tensor_copy(o_sb[:], ps[:])
                    sink(o_sb)

                idx = 0
                # c_kv chunks -> ag_in0, AllGather after the last
                for m in range(CKVT):
                    chain(
                        idx, wdkv_p, m * ET * 128, ET * 128, ET, x_mov, False,
                        lambda o, m=m: nc.scalar.dma_start(
                            ag_in0[m * 128 : (m + 1) * 128, :], o[:]
                        ),
                        w_pre=w0_sb if m == 0 else None,
                    )
                    idx += 1
                nc.gpsimd.collective_compute(
                    "AllGather",
                    mybir.AluOpType.bypass,
                    replica_groups=rg,
                    ins=[ag_in0.ap().opt()],
                    outs=[ag_out0.ap().opt()],
                )
                # c_q chunks -> SBUF resident (feature-major)
                for m in range(CQT):
                    chain(
                        idx, wdq_p, m * ET * 128, ET * 128, ET, x_mov, False, None,
                        sbuf_dst=cq_loc[:, m * 512 : (m + 1) * 512],
                    )
                    idx += 1
                # q_c chunks (all 16 heads) -> qc_a2a_in, AllToAll after last
                for m in range(QCC):
                    row = (m // 2) * 256 + (m % 2) * 128
                    chain(
                        idx, wuq_p, m * CQT * 128, CQT * 128, CQT, cq_mov, False,
                        lambda o, row=row: nc.scalar.dma_start(
                            qc_a2a_in[row : row + 128, :], o[:]
                        ),
                    )
                    idx += 1
                nc.gpsimd.collective_compute(
                    "AllToAll",
                    mybir.AluOpType.bypass,
                    replica_groups=rg,
                    ins=[qc_a2a_in.ap().opt()],
                    outs=[qc_a2a_out.ap().opt()],
                )
                # q_r chunks (rope) -> qrkr_a2a_in rows r*256..+128
                for r in range(QRC):
                    chain(
                        idx, wqr_p, r * CQT * 128, CQT * 128, CQT, cq_mov, True,
                        lambda o, r=r: nc.scalar.dma_start(
                            qrkr_a2a_in[r * 256 : r * 256 + 128, :], o[:]
                        ),
                    )
                    idx += 1
                # k_r chunks (rope) -> qrkr_a2a_in rows r*256+128..+256
                for r in range(QRC):
                    chain(
                        idx, wkr_p, r * ET * 128, ET * 128, ET, x_mov, True,
                        lambda o, r=r: nc.scalar.dma_start(
                            qrkr_a2a_in[r * 256 + 128 : r * 256 + 256, :], o[:]
                        ),
                    )
                    idx += 1
                nc.gpsimd.collective_compute(
                    "AllToAll",
                    mybir.AluOpType.bypass,
                    replica_groups=rg,
                    ins=[qrkr_a2a_in.ap().opt()],
                    outs=[qrkr_a2a_out.ap().opt()],
                )

            # ================= Phase B: heads (2 per core), both batches ====
            with (
                tc.tile_pool(name="pb_res", bufs=1) as pb_res,
                tc.tile_pool(name="pb_unit", bufs=1) as pb_unit,
                tc.tile_pool(name="pb_stream", bufs=2) as pb_stream,
                tc.tile_pool(name="pb_small", bufs=2) as pb_small,
                tc.tile_pool(name="ps_u", bufs=2, space="PSUM") as ps_u,
                tc.tile_pool(name="ps_s", bufs=2, space="PSUM") as ps_s,
                tc.tile_pool(name="ps_o", bufs=2, space="PSUM") as ps_o,
            ):
                # ---- B1: k_c and v^T for BOTH batches from gathered c_kv ----
                kc_u = {}
                vk_u = {}
                for b in range(B):
                    for h in range(HPC):
                        kc_u[b, h] = pb_unit.tile(
                            [128, S], BF16, tag=f"kc{b}{h}", bufs=1, name=f"kc{b}{h}"
                        )
                        vk_u[b, h] = pb_unit.tile(
                            [128, S], BF16, tag=f"vk{b}{h}", bufs=1, name=f"vk{b}{h}"
                        )
                for b in range(B):
                    for tbl in range(NBB):
                        tb = b * NBB + tbl
                        col = slice(tbl * 512, (tbl + 1) * 512)
                        ckv_sb = pb_stream.tile(
                            [128, CKVT * 512], BF16, tag="ckv", bufs=6, name=f"ckv_{tb}"
                        )
                        nc.scalar.dma_start(
                            ckv_sb[:].rearrange("p (c q) -> p c q", q=512),
                            ag_out0[tb * 512 : (tb + 1) * 512, :].rearrange(
                                "(c p) q -> p c q", p=128
                            ),
                        )
                        for h in range(HPC):
                            ps_kc = ps_u.tile(
                                [128, 512], F32, tag="u", bufs=2, name=f"pskc{b}{tbl}{h}"
                            )
                            for c in range(CKVT):
                                nc.tensor.matmul(
                                    ps_kc[:],
                                    wuk_sb[:, (h * CKVT + c) * 128 : (h * CKVT + c + 1) * 128],
                                    ckv_sb[:, c * 512 : (c + 1) * 512],
                                    start=(c == 0),
                                    stop=(c == CKVT - 1),
                                )
                            nc.vector.tensor_copy(kc_u[b, h][:, col], ps_kc[:])
                        for ts in range(4):
                            ps_vt = ps_u.tile(
                                [128, 512], F32, tag="u", bufs=2, name=f"psvt{b}{tbl}{ts}"
                            )
                            for c in range(CKVT):
                                nc.tensor.matmul(
                                    ps_vt[:, : HPC * HD],
                                    ckv_sb[:, c * 512 + ts * 128 : c * 512 + (ts + 1) * 128],
                                    wuvT_sb[:, c * HPC * HD : (c + 1) * HPC * HD],
                                    start=(c == 0),
                                    stop=(c == CKVT - 1),
                                )
                            for h in range(HPC):
                                nc.vector.tensor_copy(
                                    vk_u[b, h][
                                        :, tbl * 512 + ts * 128 : tbl * 512 + (ts + 1) * 128
                                    ],
                                    ps_vt[:, h * HD : (h + 1) * HD],
                                )

                # ---- attention-side q/k_r loads from the AllToAll outputs.
                # kr/qr are stored TWICE (rows 0-63 and 64-127 hold the same
                # head): the two K=64 rope-score matmuls of a pair then sit in
                # disjoint PE row-groups and execute concurrently. ----
                krd = {}
                for h in range(HPC):
                    krd[h] = pb_res.tile([128, T], BF16, tag=f"krd{h}", bufs=1, name=f"krd{h}")
                    for dup in range(2):
                        nc.scalar.dma_start(
                            krd[h][dup * 64 : (dup + 1) * 64, :].rearrange(
                                "p (j q) -> p j q", q=512
                            ),
                            qrkr_a2a_out.ap().rearrange("(j r) q -> r j q", r=256)[
                                128 + h * 64 : 128 + (h + 1) * 64, :, :
                            ],
                        )
                qc_u = {}
                qrd = {}
                for b in range(B):
                    for tbl in range(NBB):
                        tb = b * NBB + tbl
                        for h in range(HPC):
                            qrd[b, h, tbl] = pb_unit.tile(
                                [128, 512], BF16, tag=f"qrd{b}{h}{tbl}", bufs=1,
                                name=f"qrd{b}{h}{tbl}",
                            )
                            for dup in range(2):
                                nc.scalar.dma_start(
                                    qrd[b, h, tbl][dup * 64 : (dup + 1) * 64, :],
                                    qrkr_a2a_out[tb * 256 + h * 64 : tb * 256 + (h + 1) * 64, :],
                                )
                        for h in range(HPC):
                            qc_u[b, h, tbl] = pb_unit.tile(
                                [128, 512], BF16, tag=f"qc{b}{h}{tbl}", bufs=1,
                                name=f"qc{b}{h}{tbl}",
                            )
                            nc.scalar.dma_start(
                                qc_u[b, h, tbl][:],
                                qc_a2a_out[tb * 256 + h * 128 : tb * 256 + (h + 1) * 128, :],
                            )

                # ---- attention, h-major so the first head's AllToAll overlaps
                # the second head's compute ----
                for h in range(HPC):
                    hr = slice(h * RD, (h + 1) * RD)
                    for b in range(B):
                        for qb in range(NBB):
                            kmax = 4 * (qb + 1)
                            pairs = kmax // 2
                            ps_ov = ps_o.tile(
                                [128, 512], F32, tag="o", bufs=2, name=f"pso{b}{h}{qb}"
                            )

                            def issue_scores(j):
                                ps_p = ps_s.tile(
                                    [128, 1024], F32, tag="s", bufs=2,
                                    name=f"pss{b}{h}{qb}{j}",
                                )
                                for half in range(2):
                                    ki = 2 * j + half
                                    sl = slice(half * 512, (half + 1) * 512)
                                    nc.tensor.matmul(
                                        ps_p[:, sl],
                                        kc_u[b, h][:, ki * 128 : (ki + 1) * 128],
                                        qc_u[b, h, qb][:],
                                        start=True,
                                        stop=False,
                                    )
                                # the two K=64 rope matmuls run in disjoint
                                # row-groups -> concurrent on the PE array
                                for half in range(2):
                                    ki = 2 * j + half
                                    sl = slice(half * 512, (half + 1) * 512)
                                    pr = slice(half * 64, (half + 1) * 64)
                                    nc.tensor.matmul(
                                        ps_p[:, sl],
                                        krd[h][pr, b * S + ki * 128 : b * S + (ki + 1) * 128],
                                        qrd[b, h, qb][pr, :],
                                        start=False,
                                        stop=True,
                                    )
                                return ps_p

                            p_acc = pb_small.tile(
                                [128, 512], F32R, tag="pacc", bufs=2, name=f"pacc{b}{h}{qb}"
                            )
                            ps_p = issue_scores(0)
                            for j in range(pairs):
                                ps_nxt = issue_scores(j + 1) if j + 1 < pairs else None
                                p_sb = pb_small.tile(
                                    [128, 1024], BF16, tag="p", bufs=3, name=f"p{b}{h}{qb}{j}"
                                )
                                nc.scalar.activation(
                                    p_sb[:], ps_p[:], AF.Exp, scale=float(SCALE)
                                )
                                for half in range(2):
                                    ki = 2 * j + half
                                    if ki >= 4 * qb:
                                        o = ki - 4 * qb
                                        nc.vector.tensor_mul(
                                            p_sb[:, half * 512 : (half + 1) * 512],
                                            p_sb[:, half * 512 : (half + 1) * 512],
                                            mask_sb[:, o * 512 : (o + 1) * 512],
                                        )
                                if j == 0:
                                    nc.vector.tensor_tensor(
                                        p_acc[:], p_sb[:, :512], p_sb[:, 512:],
                                        op=mybir.AluOpType.add,
                                    )
                                else:
                                    for half in range(2):
                                        nc.vector.tensor_tensor(
                                            p_acc[:], p_acc[:],
                                            p_sb[:, half * 512 : (half + 1) * 512],
                                            op=mybir.AluOpType.add,
                                        )
                                for half in range(2):
                                    ki = 2 * j + half
                                    qlo = max(0, (ki - 4 * qb) * 128)
                                    nc.tensor.matmul(
                                        ps_ov[:, qlo:],
                                        vk_u[b, h][:, ki * 128 : (ki + 1) * 128],
                                        p_sb[:, half * 512 + qlo : (half + 1) * 512],
                                        start=(ki == 0),
                                        stop=(ki == kmax - 1),
                                    )
                                ps_p = ps_nxt
                            ps_den = ps_s.tile(
                                [128, 1024], F32, tag="s", bufs=2, name=f"psd{b}{h}{qb}"
                            )
                            nc.tensor.matmul(
                                ps_den[:, :512], ones_r[:], p_acc[:], start=True, stop=True
                            )
                            rc_sb = pb_small.tile(
                                [128, 512], F32, tag="dn", bufs=2, name=f"dn{b}{h}{qb}"
                            )
                            nc.vector.reciprocal_approx_fast(rc_sb[:], ps_den[:, :512])
                            o_sb = pb_small.tile(
                                [128, 512], BF16, tag="os", bufs=2, name=f"os{b}{h}{qb}"
                            )
                            nc.vector.tensor_mul(o_sb[:], ps_ov[:], rc_sb[:])
                            row = (b * NBB + qb) * HD
                            nc.sync.dma_start(a2a_in[h][row : row + HD, :], o_sb[:])
                    # all (b, qb) outputs for this head are written; fire its
                    # AllToAll so it overlaps the next head's compute
                    nc.gpsimd.collective_compute(
                        "AllToAll",
                        mybir.AluOpType.bypass,
                        replica_groups=rg,
                        ins=[a2a_in[h].ap().opt()],
                        outs=[a2a_out[h].ap().opt()],
                    )

                # ============ Phase C: out-projection, 2-stage so the first
                # half (head-0 dims, available after the first AllToAll)
                # overlaps the second head's attention ============
                of_half = []
                for h in range(HPC):
                    ofh = pb_unit.tile([128, 8 * 512], BF16, tag=f"of{h}", bufs=1, name=f"of{h}")
                    eng = nc.sync if h == 0 else nc.gpsimd
                    eng.dma_start(
                        ofh[:].rearrange("p (d q) -> p d q", q=512),
                        a2a_out[h].ap().rearrange("(d p) q -> p d q", p=128),
                    )
                    of_half.append(ofh)
                # head-0 partials stay resident in SBUF — no DRAM bounce
                oc_sb = pb_res.tile([128, ET * 512], BF16, tag="ocs", bufs=1, name="ocs")
                for ec in range(ET):
                    wo_sb = pb_stream.tile([128, 8 * 128], BF16, tag="wo", bufs=6, name=f"wo0_{ec}")
                    nc.sync.dma_start(
                        wo_sb[:], wout_p[:, ec * ET * 128 : ec * ET * 128 + 8 * 128]
                    )
                    ps = ps_u.tile([128, 512], F32, tag="u", bufs=2, name=f"psca{ec}")
                    for d in range(8):
                        nc.tensor.matmul(
                            ps[:],
                            wo_sb[:, d * 128 : (d + 1) * 128],
                            of_half[0][:, d * 512 : (d + 1) * 512],
                            start=(d == 0),
                            stop=(d == 7),
                        )
                    nc.vector.tensor_copy(oc_sb[:, ec * 512 : (ec + 1) * 512], ps[:])
                for ec in range(ET):
                    wo_sb = pb_stream.tile([128, 8 * 128], BF16, tag="wo", bufs=6, name=f"wo1_{ec}")
                    nc.sync.dma_start(
                        wo_sb[:], wout_p[:, ec * ET * 128 + 8 * 128 : (ec + 1) * ET * 128]
                    )
                    ps = ps_u.tile([128, 512], F32, tag="u", bufs=2, name=f"pscb{ec}")
                    for d in range(8):
                        nc.tensor.matmul(
                            ps[:],
                            wo_sb[:, d * 128 : (d + 1) * 128],
                            of_half[1][:, d * 512 : (d + 1) * 512],
                            start=(d == 0),
                            stop=(d == 7),
                        )
                    o_sb = pb_small.tile([128, 512], F32, tag="ocf", bufs=2, name=f"ocf{ec}")
                    nc.vector.tensor_tensor(
                        o_sb[:], ps[:], oc_sb[:, ec * 512 : (ec + 1) * 512],
                        op=mybir.AluOpType.add,
                    )
                    nc.sync.dma_start(out_t[ec * 128 : (ec + 1) * 128, :], o_sb[:])

    nc.compile()
    return nc


_NC_CACHE = None


def _get_program():
    global _NC_CACHE
    if _NC_CACHE is None:
        _NC_CACHE = build_program()
    return _NC_CACHE


def _host_tables():
    pos = np.arange(S, dtype=np.float32)
    inv_freq = 1.0 / (10000.0 ** (np.arange(0, RD, 2, dtype=np.float32) / RD))
    freqs = pos[:, None] * inv_freq[None, :]          # [S, 32]
    cos64 = np.concatenate([np.cos(freqs)] * 2, axis=1).T.astype(np.float32)  # [64, S]
    sin64 = np.sin(freqs).T.astype(np.float32)        # [32, S]
    sin_signed = np.concatenate([-sin64, sin64], axis=0)  # [64, S]
    cos_full = np.tile(cos64, (2, 1))                 # [128, S]
    sin_full = np.tile(sin_signed, (2, 1))            # [128, S]
    kk = np.arange(128)[:, None]
    qq = np.arange(512)[None, :]
    mask = np.concatenate(
        [(kk + o * 128 <= qq).astype(np.float32) for o in range(4)], axis=1
    ).astype(ml_dtypes.bfloat16)                      # [128, 2048]
    return cos_full, sin_full, mask


def _pack_pm(w_t, n_in_tiles, n_out):
    """Pack [n_in_tiles*128, n_out] so chunk m is [128, n_in_tiles, 128] with
    long contiguous partition rows: out[p, ((m*n_in_tiles)+e)*128+f] = w_t[e*128+p, m*128+f]."""
    n_chunks = n_out // 128
    a = w_t.reshape(n_in_tiles, 128, n_chunks, 128).transpose(1, 2, 0, 3)
    return np.ascontiguousarray(a.reshape(128, n_chunks * n_in_tiles * 128))


def kernel(x, w_dq, w_uq, w_dkv, w_uk, w_uv, w_qr, w_kr, w_out):
    x = np.asarray(x, dtype=np.float32)
    w_dq = np.asarray(w_dq, dtype=np.float32)
    w_uq = np.asarray(w_uq, dtype=np.float32)
    w_dkv = np.asarray(w_dkv, dtype=np.float32)
    w_uk = np.asarray(w_uk, dtype=np.float32)
    w_uv = np.asarray(w_uv, dtype=np.float32)
    w_qr = np.asarray(w_qr, dtype=np.float32)
    w_kr = np.asarray(w_kr, dtype=np.float32)
    w_out = np.asarray(w_out, dtype=np.float32)

    nc = _get_program()
    cos_full, sin_full, mask = _host_tables()

    xt = np.ascontiguousarray(x.reshape(T, E).T)          # [E, T]
    wdq_p = _pack_pm(w_dq.T, ET, CQ).astype(ml_dtypes.bfloat16)
    wdkv_p = _pack_pm(w_dkv.T, ET, CKV).astype(ml_dtypes.bfloat16)
    wuq_p = _pack_pm(w_uq.T, CQT, H * HD).astype(ml_dtypes.bfloat16)
    wqr_p = _pack_pm(w_qr.T, CQT, H * RD).astype(ml_dtypes.bfloat16)
    wkr_p = _pack_pm(w_kr.T, ET, H * RD).astype(ml_dtypes.bfloat16)
    # permute w_out's input-dim tiles to [even heads, odd heads] to match the
    # head-split AllToAll reassembly in phase C
    perm = [2 * j for j in range(8)] + [2 * j + 1 for j in range(8)]
    wout_perm = w_out.T.reshape(ET, 128, E)[perm].reshape(E, E)
    wout_p = _pack_pm(wout_perm, ET, E).astype(ml_dtypes.bfloat16)
    ones_f32 = np.ones((128, 128), dtype=np.float32)

    in_maps = []
    for i in range(NC):
        hp = slice(i * HPC * HD, (i + 1) * HPC * HD)      # this core's head dims
        xt_loc = xt[:, i * TPC : (i + 1) * TPC]
        x_pi = np.ascontiguousarray(
            xt_loc.reshape(ET, 128, TPC).transpose(1, 0, 2).reshape(128, ET * TPC)
        ).astype(ml_dtypes.bfloat16)
        pos0 = (i % NBB) * 512
        in_maps.append(
            {
                "x_p": x_pi,
                "wdq_p": wdq_p,
                "wdkv_p": wdkv_p,
                "wkr_p": wkr_p,
                "wuq_p": wuq_p,
                "wqr_p": wqr_p,
                "wuk_p": _pack_pm(w_uk[hp, :].T, CKVT, HPC * HD).astype(ml_dtypes.bfloat16),
                # transposed pack for the direct-v^T matmuls:
                # wuv_p[p, c*256+hh] = w_uv[hp hh, c*128+p]
                "wuv_p": np.ascontiguousarray(
                    w_uv[hp, :].T.reshape(CKVT, 128, HPC * HD)
                    .transpose(1, 0, 2)
                    .reshape(128, CKVT * HPC * HD)
                ).astype(ml_dtypes.bfloat16),
                "wout_p": wout_p,
                "cos_t": np.ascontiguousarray(cos_full[:, pos0 : pos0 + 512]),
                "sin_t": np.ascontiguousarray(sin_full[:, pos0 : pos0 + 512]),
                "mask_t": mask,
                "ones_f32_t": ones_f32,
            }
        )

    res = bass_utils.run_bass_kernel_spmd(nc, in_maps, core_ids=list(range(NC)))
    out = np.concatenate(
        [np.ascontiguousarray(res.results[i]["out_t"].T) for i in range(NC)], axis=0
    )
    return out.reshape(B, S, E)


def run_profiled(inputs):
    """Used by test.py: run once with NTFF tracing, return (output, exec_time_ns)."""
    sys.path.insert(0, "/root/.axon_site")
    from trn_agent_boot.trn_boot import _ntff_profile_via_ctypes

    hooks_mod = types.ModuleType("antenv.axon_hooks")
    hook = _ntff_profile_via_ctypes("/opt/axon/libaxon_pjrt.so")
    hooks_mod.get_axon_ntff_profile_hook = lambda: hook
    sys.modules["antenv.axon_hooks"] = hooks_mod

    orig = bass_utils.run_bass_kernel_spmd
    holder = {}

    def wrapper(nc, in_maps, core_ids, **kw):
        kw["trace"] = True
        res = orig(nc, in_maps, core_ids, **kw)
        holder["exec_time_ns"] = res.exec_time_ns
        return res

    bass_utils.run_bass_kernel_spmd = wrapper
    try:
        out = kernel(**inputs)
    finally:
        bass_utils.run_bass_kernel_spmd = orig
    return out, holder.get("exec_time_ns")



# revision 7
# speedup vs baseline: 1.2898x; 1.2898x over previous
"""Multi-head latent attention (MLA) prefill kernel for 8 Trainium2 NeuronCores.

Sharding strategy (token-parallel projections + head-parallel attention):
  Phase A (token-parallel): each core computes, for ITS 512 of the 4096
    tokens, the compressed latents c_q/c_kv AND the full q_c/q_r/k_r
    up-projections for ALL 16 heads. c_kv is AllGathered (4.2MB) for the
    k/v up-projections; q_c and q_r/k_r are exchanged with two 2.1MB
    AllToAlls that land each core its 2 heads for all 4096 tokens.
  Phase B (head-parallel): each core up-projects k_c and v^T for its 2
    heads from the gathered c_kv, then runs causal attention.
  Phase C: an AllToAll token-shards the attention output O so each core
    applies the full out-projection to its 512 tokens.

Schedule notes (v2):
  - Phase A order is ckv -> AllGather -> c_q -> q_c -> qc A2A -> q_r ->
    k_r -> qrkr A2A so each collective fires as early as its inputs allow.
  - All PSUM->SBUF drains in phases A/B1/C run on the scalar (ACT) engine,
    which is otherwise idle there; the vector engine only does rope math
    (in bf16, SBUF-to-SBUF, which hits the DVE 2-byte fast path).
  - B-phase input loads are issued on the gpsimd queue in data-readiness
    order (ckv after AllGather, qc_u after the qc A2A, krd/qrd after the
    qrkr A2A) so no load blocks an earlier-ready one.
  - Attention probability path is fp16: exp writes fp16, mask multiplies
    and denominator accumulation run at the DVE 2-byte rate, and the
    denominator row-sum is one fp16 ones-matmul (vs f32r at 4x cost).

Attention inner loop works on PAIRS of 128-key blocks: scores for pair j+1
are issued before the ov matmuls of pair j, exp runs once per pair on a
2-bank [128,1024] PSUM tile (scores are ~N(0,1) after scaling so exp cannot
overflow). v^T is computed directly (stationary = c_kv token block, moving
= w_uv^T both heads) instead of v followed by PE transposes. kr/qr rope
rows are stored twice so the two K=64 rope-score matmuls of a pair sit in
disjoint PE row-groups and execute concurrently.

Weight streams are host-packed so each tile load is a single DMA with long
contiguous runs. Out-projection head-0 partials stay resident in SBUF.
"""

import sys
import types

sys.path.insert(0, "/opt/trn_rl_repo")

import ml_dtypes
import numpy as np

from concourse import bacc, bass, mybir, tile
from concourse import bass_utils

F32 = mybir.dt.float32
BF16 = mybir.dt.bfloat16
F16 = mybir.dt.float16
AF = mybir.ActivationFunctionType

E = 2048
H = 16
HD = 128
CKV = 512
CQ = 1536
RD = 64
SCALE = 1.0 / np.sqrt(HD + RD)
B = 2
S = 2048
T = B * S            # 4096 tokens
NC = 8               # cores
TPC = T // NC        # 512 tokens per core
HPC = H // NC        # 2 heads per core
NB = T // 512        # 8 token blocks of 512
NBB = S // 512       # 4 token blocks per batch
ET = E // 128        # 16 e-tiles
CQT = CQ // 128      # 12 c_q tiles
CKVT = CKV // 128    # 4 c_kv tiles
QCC = H              # 16 q_c output chunks of 128 dims
QRC = H * RD // 128  # 8 q_r / k_r output chunks of 128 dims


def build_program():
    nc = bacc.Bacc("TRN2", target_bir_lowering=False, debug=False, num_devices=NC)

    # ---- I/O ----
    # *_p tensors are host-packed to [128 partitions, ...] so tile loads are
    # single DMAs with long contiguous runs.
    x_p = nc.dram_tensor("x_p", [128, ET * TPC], BF16, kind="ExternalInput")
    wdq_p = nc.dram_tensor("wdq_p", [128, CQT * ET * 128], BF16, kind="ExternalInput")
    wdkv_p = nc.dram_tensor("wdkv_p", [128, CKVT * ET * 128], BF16, kind="ExternalInput")
    wkr_p = nc.dram_tensor("wkr_p", [128, QRC * ET * 128], BF16, kind="ExternalInput")
    wuq_p = nc.dram_tensor("wuq_p", [128, QCC * CQT * 128], BF16, kind="ExternalInput")
    wqr_p = nc.dram_tensor("wqr_p", [128, QRC * CQT * 128], BF16, kind="ExternalInput")
    wuk_p = nc.dram_tensor("wuk_p", [128, CKVT * 256], BF16, kind="ExternalInput")
    wuv_p = nc.dram_tensor("wuv_p", [128, CKVT * 256], BF16, kind="ExternalInput")
    wout_p = nc.dram_tensor("wout_p", [128, ET * ET * 128], BF16, kind="ExternalInput")
    cos_t = nc.dram_tensor("cos_t", [128, 512], BF16, kind="ExternalInput")
    sin_t = nc.dram_tensor("sin_t", [128, 512], BF16, kind="ExternalInput")
    mask_t = nc.dram_tensor("mask_t", [128, 4 * 512], F16, kind="ExternalInput")
    ones_t = nc.dram_tensor("ones_t", [128, 128], F16, kind="ExternalInput")
    out_t = nc.dram_tensor("out_t", [E, TPC], F32, kind="ExternalOutput")

    # ---- internal DRAM (collective bounce buffers) ----
    ag_in0 = nc.dram_tensor("ag_in0", [CKV, TPC], BF16)
    ag_out0 = nc.dram_tensor("ag_out0", [NC * CKV, TPC], BF16, addr_space="Shared")
    # shard j of qc: q_c rows for heads 2j/2j+1 (my tokens);
    # shard j of qrkr: q_r rows (128) then k_r rows (128) for heads 2j/2j+1.
    qc_a2a_in = nc.dram_tensor("qc_a2a_in", [NC * 256, 512], BF16)
    qc_a2a_out = nc.dram_tensor("qc_a2a_out", [NC * 256, 512], BF16)
    qrkr_a2a_in = nc.dram_tensor("qrkr_a2a_in", [NC * 256, 512], BF16)
    qrkr_a2a_out = nc.dram_tensor("qrkr_a2a_out", [NC * 256, 512], BF16)
    a2a_in = [nc.dram_tensor(f"a2a_in{h}", [NC * HD, 512], BF16) for h in range(HPC)]
    a2a_out = [nc.dram_tensor(f"a2a_out{h}", [NC * HD, 512], BF16) for h in range(HPC)]

    rg = [list(range(NC))]

    with tile.TileContext(nc) as tc:
        with (
            tc.tile_pool(name="pc_const", bufs=1) as pc_const,
            tc.tile_pool(name="pc_small", bufs=2) as pc_small,
            tc.tile_pool(name="pb_ckv", bufs=1) as pb_ckv,
            tc.tile_pool(name="pb_unit", bufs=1) as pb_unit,
            tc.tile_pool(name="pb_res", bufs=1) as pb_res,
        ):
            # constants resident for the whole kernel; loaded on the vector
            # queue so the first weight/x DMAs on sync are not delayed
            mask_sb = pc_const.tile([128, 4 * 512], F16)
            nc.gpsimd.dma_start(mask_sb[:], mask_t[:, :])
            wuk_sb = pc_const.tile([128, CKVT * 256], BF16)
            nc.gpsimd.dma_start(wuk_sb[:], wuk_p[:, :])
            # wuv packed transposed: wuvT_sb[p, c*256+hh] = w_uv[hp hh, c*128+p]
            wuvT_sb = pc_const.tile([128, CKVT * 256], BF16)
            nc.gpsimd.dma_start(wuvT_sb[:], wuv_p[:, :])
            ones_sb = pc_const.tile([128, 128], F16)
            nc.gpsimd.dma_start(ones_sb[:], ones_t[:, :])
            cos_sb = pc_const.tile([128, 512], BF16)
            nc.gpsimd.dma_start(cos_sb[:], cos_t[:, :])
            sin_sb = pc_const.tile([128, 512], BF16)
            nc.gpsimd.dma_start(sin_sb[:], sin_t[:, :])

            def rope_local(dst, rp):
                """dst[:, 512] = rope(rp[:, 512]), all bf16 in SBUF.

                Rows are 64-dim RoPE blocks (one per head); rotate-half pairs
                row d with d+32 inside each block. sin comes pre-signed.
                """
                sh = pc_small.tile([128, 512], BF16, tag="sh", bufs=2, name="sh")
                for blk in range(2):
                    p0 = blk * 64
                    nc.vector.tensor_copy(sh[p0 : p0 + 32, :], rp[p0 + 32 : p0 + 64, :])
                    nc.vector.tensor_copy(sh[p0 + 32 : p0 + 64, :], rp[p0 : p0 + 32, :])
                t1 = pc_small.tile([128, 512], BF16, tag="t1", bufs=2, name="t1")
                nc.vector.tensor_mul(t1[:], rp[:], cos_sb[:])
                nc.vector.tensor_mul(sh[:], sh[:], sin_sb[:])
                nc.vector.tensor_add(dst, t1[:], sh[:])

            # B-phase tiles whose loads are issued early on the gpsimd queue
            ckv_sb = {}
            for b in range(B):
                for tbl in range(NBB):
                    tb = b * NBB + tbl
                    ckv_sb[b, tbl] = pb_ckv.tile(
                        [128, CKVT * 512], BF16, tag=f"ckv{tb}", bufs=1, name=f"ckv_{tb}"
                    )
            qc_u = {}
            qrd = {}
            for b in range(B):
                for tbl in range(NBB):
                    for h in range(HPC):
                        qc_u[b, h, tbl] = pb_unit.tile(
                            [128, 512], BF16, tag=f"qc{b}{h}{tbl}", bufs=1,
                            name=f"qc{b}{h}{tbl}",
                        )
                        qrd[b, h, tbl] = pb_unit.tile(
                            [128, 512], BF16, tag=f"qrd{b}{h}{tbl}", bufs=1,
                            name=f"qrd{b}{h}{tbl}",
                        )
            krd = {}
            for h in range(HPC):
                krd[h] = pb_res.tile([128, T], BF16, tag=f"krd{h}", bufs=1, name=f"krd{h}")

            # ============ Phase A: token-local projections (my 512 tokens) ==
            with (
                tc.tile_pool(name="pa_x", bufs=1) as pa_x,
                tc.tile_pool(name="pa_we", bufs=4) as pa_we,
                tc.tile_pool(name="pa_wc", bufs=4) as pa_wc,
                tc.tile_pool(name="pa_s", bufs=3) as pa_s,
                tc.tile_pool(name="pa_rp", bufs=2) as pa_rp,
                tc.tile_pool(name="pa_loc", bufs=1) as pa_loc,
                tc.tile_pool(name="pa_ps", bufs=4, space="PSUM") as pa_ps,
            ):
                # first weight chunk is issued before the x stream so the
                # first chain's LDWEIGHTS isn't stuck behind 2MB of x
                w0_sb = pa_we.tile([128, ET * 128], BF16, tag="we", bufs=4, name="w_pre")
                nc.sync.dma_start(w0_sb[:], wdkv_p[:, : ET * 128])
                x_half = []
                for xh in range(2):
                    xt_ = pa_x.tile([128, 8 * TPC], BF16, tag=f"x{xh}", bufs=1, name=f"x{xh}")
                    # quarter-chunked so the first chain starts ~1 chunk in
                    for qtr in range(2):
                        nc.sync.dma_start(
                            xt_[:, qtr * 4 * TPC : (qtr + 1) * 4 * TPC],
                            x_p[:, (xh * 8 + qtr * 4) * TPC : (xh * 8 + (qtr + 1) * 4) * TPC],
                        )
                    x_half.append(xt_)
                cq_loc = pa_loc.tile([128, CQT * 512], BF16, tag="cq", bufs=1, name="cq_loc")

                def x_mov(e):
                    return x_half[e // 8][:, (e % 8) * TPC : (e % 8 + 1) * TPC]

                def cq_mov(c):
                    return cq_loc[:, c * 512 : (c + 1) * 512]

                def chain(idx, w_dram, w_off, w_cols, nmm, mov, rope, sink, sbuf_dst=None, w_pre=None):
                    """one output chunk: stream weight tile, accumulate, emit"""
                    if w_pre is not None:
                        w_sb = w_pre
                    else:
                        wtag, wpool, wb = ("we", pa_we, 4) if w_cols == ET * 128 else ("wc", pa_wc, 4)
                        w_sb = wpool.tile([128, w_cols], BF16, tag=wtag, bufs=wb, name=f"w{idx}")
                        nc.sync.dma_start(w_sb[:], w_dram[:, w_off : w_off + w_cols])
                    ps = pa_ps.tile([128, 512], F32, tag="pa", bufs=4, name=f"ps{idx}")
                    for k in range(nmm):
                        nc.tensor.matmul(
                            ps[:],
                            w_sb[:, k * 128 : (k + 1) * 128],
                            mov(k),
                            start=(k == 0),
                            stop=(k == nmm - 1),
                        )
                    if sbuf_dst is not None:
                        nc.scalar.copy(sbuf_dst, ps[:])
                        return
                    o_sb = pa_s.tile([128, 512], BF16, tag="oa", bufs=3, name=f"oa{idx}")
                    if rope:
                        rp_sb = pa_rp.tile([128, 512], BF16, tag="rp", bufs=2, name=f"rp{idx}")
                        nc.scalar.copy(rp_sb[:], ps[:])
                        rope_local(o_sb[:], rp_sb)
                    else:
                        nc.scalar.copy(o_sb[:], ps[:])
                    sink(o_sb)

                idx = 0
                # c_kv chunks -> ag_in0, AllGather after the last
                for m in range(CKVT):
                    chain(
                        idx, wdkv_p, m * ET * 128, ET * 128, ET, x_mov, False,
                        lambda o, m=m: nc.scalar.dma_start(
                            ag_in0[m * 128 : (m + 1) * 128, :], o[:]
                        ),
                        w_pre=w0_sb if m == 0 else None,
                    )
                    idx += 1
                nc.gpsimd.collective_compute(
                    "AllGather",
                    mybir.AluOpType.bypass,
                    replica_groups=rg,
                    ins=[ag_in0.ap().opt()],
                    outs=[ag_out0.ap().opt()],
                )
                # B1 input loads right behind the AllGather on the gpsimd
                # queue: they run as soon as ag_out0 is ready
                for b in range(B):
                    for tbl in range(NBB):
                        tb = b * NBB + tbl
                        nc.gpsimd.dma_start(
                            ckv_sb[b, tbl][:].rearrange("p (c q) -> p c q", q=512),
                            ag_out0[tb * 512 : (tb + 1) * 512, :].rearrange(
                                "(c p) q -> p c q", p=128
                            ),
                        )
                # c_q chunks -> SBUF resident (feature-major)
                for m in range(CQT):
                    chain(
                        idx, wdq_p, m * ET * 128, ET * 128, ET, x_mov, False, None,
                        sbuf_dst=cq_loc[:, m * 512 : (m + 1) * 512],
                    )
                    idx += 1
                # q_c chunks (all 16 heads) -> qc_a2a_in, AllToAll after last
                for m in range(QCC):
                    row = (m // 2) * 256 + (m % 2) * 128
                    chain(
                        idx, wuq_p, m * CQT * 128, CQT * 128, CQT, cq_mov, False,
                        lambda o, row=row: nc.scalar.dma_start(
                            qc_a2a_in[row : row + 128, :], o[:]
                        ),
                    )
                    idx += 1
                nc.gpsimd.collective_compute(
                    "AllToAll",
                    mybir.AluOpType.bypass,
                    replica_groups=rg,
                    ins=[qc_a2a_in.ap().opt()],
                    outs=[qc_a2a_out.ap().opt()],
                )
                # qc_u loads right behind their A2A
                for b in range(B):
                    for tbl in range(NBB):
                        tb = b * NBB + tbl
                        for h in range(HPC):
                            nc.gpsimd.dma_start(
                                qc_u[b, h, tbl][:],
                                qc_a2a_out[tb * 256 + h * 128 : tb * 256 + (h + 1) * 128, :],
                            )
                # q_r chunks (rope) -> qrkr_a2a_in rows r*256..+128
                for r in range(QRC):
                    chain(
                        idx, wqr_p, r * CQT * 128, CQT * 128, CQT, cq_mov, True,
                        lambda o, r=r: nc.scalar.dma_start(
                            qrkr_a2a_in[r * 256 : r * 256 + 128, :], o[:]
                        ),
                    )
                    idx += 1
                # k_r chunks (rope) -> qrkr_a2a_in rows r*256+128..+256
                for r in range(QRC):
                    chain(
                        idx, wkr_p, r * ET * 128, ET * 128, ET, x_mov, True,
                        lambda o, r=r: nc.scalar.dma_start(
                            qrkr_a2a_in[r * 256 + 128 : r * 256 + 256, :], o[:]
                        ),
                    )
                    idx += 1
                nc.gpsimd.collective_compute(
                    "AllToAll",
                    mybir.AluOpType.bypass,
                    replica_groups=rg,
                    ins=[qrkr_a2a_in.ap().opt()],
                    outs=[qrkr_a2a_out.ap().opt()],
                )

            # ---- attention-side q/k_r loads from the AllToAll outputs.
            # kr/qr are stored TWICE (rows 0-63 and 64-127 hold the same
            # head): the two K=64 rope-score matmuls of a pair then sit in
            # disjoint PE row-groups and execute concurrently. ----
            for h in range(HPC):
                for dup in range(2):
                    nc.gpsimd.dma_start(
                        krd[h][dup * 64 : (dup + 1) * 64, :].rearrange(
                            "p (j q) -> p j q", q=512
                        ),
                        qrkr_a2a_out.ap().rearrange("(j r) q -> r j q", r=256)[
                            128 + h * 64 : 128 + (h + 1) * 64, :, :
                        ],
                    )
            for b in range(B):
                for tbl in range(NBB):
                    tb = b * NBB + tbl
                    for h in range(HPC):
                        for dup in range(2):
                            nc.gpsimd.dma_start(
                                qrd[b, h, tbl][dup * 64 : (dup + 1) * 64, :],
                                qrkr_a2a_out[tb * 256 + h * 64 : tb * 256 + (h + 1) * 64, :],
                            )

            # ================= Phase B: heads (2 per core), both batches ====
            with (
                tc.tile_pool(name="pb_stream", bufs=2) as pb_stream,
                tc.tile_pool(name="pb_small", bufs=2) as pb_small,
                tc.tile_pool(name="pb_kv", bufs=1) as pb_kv,
            ):
                # ---- B1: k_c and v^T for BOTH batches from gathered c_kv ----
                kc_u = {}
                vk_u = {}
                for b in range(B):
                    for h in range(HPC):
                        kc_u[b, h] = pb_kv.tile(
                            [128, S], BF16, tag=f"kc{b}{h}", bufs=1, name=f"kc{b}{h}"
                        )
                        vk_u[b, h] = pb_kv.tile(
                            [128, S], F16, tag=f"vk{b}{h}", bufs=1, name=f"vk{b}{h}"
                        )
                with tc.tile_pool(name="ps_b1", bufs=2, space="PSUM") as ps_b1:
                    for b in range(B):
                        for tbl in range(NBB):
                            col = slice(tbl * 512, (tbl + 1) * 512)
                            for h in range(HPC):
                                ps_kc = ps_b1.tile(
                                    [128, 512], F32, tag="u", bufs=2, name=f"pskc{b}{tbl}{h}"
                                )
                                for c in range(CKVT):
                                    nc.tensor.matmul(
                                        ps_kc[:],
                                        wuk_sb[:, (h * CKVT + c) * 128 : (h * CKVT + c + 1) * 128],
                                        ckv_sb[b, tbl][:, c * 512 : (c + 1) * 512],
                                        start=(c == 0),
                                        stop=(c == CKVT - 1),
                                    )
                                nc.scalar.copy(kc_u[b, h][:, col], ps_kc[:])
                            for ts in range(4):
                                ps_vt = ps_b1.tile(
                                    [128, 512], F32, tag="u", bufs=2, name=f"psvt{b}{tbl}{ts}"
                                )
                                for c in range(CKVT):
                                    nc.tensor.matmul(
                                        ps_vt[:, : HPC * HD],
                                        ckv_sb[b, tbl][:, c * 512 + ts * 128 : c * 512 + (ts + 1) * 128],
                                        wuvT_sb[:, c * HPC * HD : (c + 1) * HPC * HD],
                                        start=(c == 0),
                                        stop=(c == CKVT - 1),
                                    )
                                for h in range(HPC):
                                    nc.scalar.copy(
                                        vk_u[b, h][
                                            :, tbl * 512 + ts * 128 : tbl * 512 + (ts + 1) * 128
                                        ],
                                        ps_vt[:, h * HD : (h + 1) * HD],
                                    )

                # ---- attention, h-major so the first head's AllToAll overlaps
                # the second head's compute ----
                of_half = []
                with (
                    tc.tile_pool(name="ps_s", bufs=2, space="PSUM") as ps_s,
                    tc.tile_pool(name="ps_o", bufs=2, space="PSUM") as ps_o,
                    tc.tile_pool(name="ps_d", bufs=2, space="PSUM") as ps_d,
                ):
                    for h in range(HPC):
                        for b in range(B):
                            for qb in range(NBB):
                                kmax = 4 * (qb + 1)
                                pairs = kmax // 2
                                ps_ov = ps_o.tile(
                                    [128, 512], F32, tag="o", bufs=2, name=f"pso{b}{h}{qb}"
                                )

                                def issue_scores(j):
                                    ps_p = ps_s.tile(
                                        [128, 1024], F32, tag="s", bufs=2,
                                        name=f"pss{b}{h}{qb}{j}",
                                    )
                                    for half in range(2):
                                        ki = 2 * j + half
                                        sl = slice(half * 512, (half + 1) * 512)
                                        nc.tensor.matmul(
                                            ps_p[:, sl],
                                            kc_u[b, h][:, ki * 128 : (ki + 1) * 128],
                                            qc_u[b, h, qb][:],
                                            start=True,
                                            stop=False,
                                        )
                                    # the two K=64 rope matmuls run in disjoint
                                    # row-groups -> concurrent on the PE array
                                    for half in range(2):
                                        ki = 2 * j + half
                                        sl = slice(half * 512, (half + 1) * 512)
                                        pr = slice(half * 64, (half + 1) * 64)
                                        nc.tensor.matmul(
                                            ps_p[:, sl],
                                            krd[h][pr, b * S + ki * 128 : b * S + (ki + 1) * 128],
                                            qrd[b, h, qb][pr, :],
                                            start=False,
                                            stop=True,
                                        )
                                    return ps_p

                                p_acc = pb_small.tile(
                                    [128, 512], F16, tag="pacc", bufs=2, name=f"pacc{b}{h}{qb}"
                                )
                                ps_p = issue_scores(0)
                                for j in range(pairs):
                                    ps_nxt = issue_scores(j + 1) if j + 1 < pairs else None
                                    p_sb = pb_small.tile(
                                        [128, 1024], F16, tag="p", bufs=3, name=f"p{b}{h}{qb}{j}"
                                    )
                                    nc.scalar.activation(
                                        p_sb[:], ps_p[:], AF.Exp, scale=float(SCALE)
                                    )
                                    for half in range(2):
                                        ki = 2 * j + half
                                        if ki >= 4 * qb:
                                            o = ki - 4 * qb
                                            nc.vector.tensor_mul(
                                                p_sb[:, half * 512 : (half + 1) * 512],
                                                p_sb[:, half * 512 : (half + 1) * 512],
                                                mask_sb[:, o * 512 : (o + 1) * 512],
                                            )
                                    if j == 0:
                                        nc.vector.tensor_tensor(
                                            p_acc[:], p_sb[:, :512], p_sb[:, 512:],
                                            op=mybir.AluOpType.add,
                                        )
                                    else:
                                        for half in range(2):
                                            nc.vector.tensor_tensor(
                                                p_acc[:], p_acc[:],
                                                p_sb[:, half * 512 : (half + 1) * 512],
                                                op=mybir.AluOpType.add,
                                            )
                                    for half in range(2):
                                        ki = 2 * j + half
                                        qlo = max(0, (ki - 4 * qb) * 128)
                                        nc.tensor.matmul(
                                            ps_ov[:, qlo:],
                                            vk_u[b, h][:, ki * 128 : (ki + 1) * 128],
                                            p_sb[:, half * 512 + qlo : (half + 1) * 512],
                                            start=(ki == 0),
                                            stop=(ki == kmax - 1),
                                        )
                                    ps_p = ps_nxt
                                ps_den = ps_d.tile(
                                    [128, 512], F32, tag="d", bufs=2, name=f"psd{b}{h}{qb}"
                                )
                                nc.tensor.matmul(
                                    ps_den[:], ones_sb[:], p_acc[:], start=True, stop=True
                                )
                                rc_sb = pb_small.tile(
                                    [128, 512], F32, tag="dn", bufs=2, name=f"dn{b}{h}{qb}"
                                )
                                nc.vector.reciprocal_approx_fast(rc_sb[:], ps_den[:])
                                o_sb = pb_small.tile(
                                    [128, 512], BF16, tag="os", bufs=2, name=f"os{b}{h}{qb}"
                                )
                                nc.vector.tensor_mul(o_sb[:], ps_ov[:], rc_sb[:])
                                row = (b * NBB + qb) * HD
                                nc.sync.dma_start(a2a_in[h][row : row + HD, :], o_sb[:])
                        # all (b, qb) outputs for this head are written; fire its
                        # AllToAll so it overlaps the next head's compute
                        nc.gpsimd.collective_compute(
                            "AllToAll",
                            mybir.AluOpType.bypass,
                            replica_groups=rg,
                            ins=[a2a_in[h].ap().opt()],
                            outs=[a2a_out[h].ap().opt()],
                        )
                        # gather this head's out-proj input right behind its A2A
                        ofh = pb_res.tile(
                            [128, 8 * 512], BF16, tag=f"of{h}", bufs=1, name=f"of{h}"
                        )
                        nc.gpsimd.dma_start(
                            ofh[:].rearrange("p (d q) -> p d q", q=512),
                            a2a_out[h].ap().rearrange("(d p) q -> p d q", p=128),
                        )
                        of_half.append(ofh)

                # ============ Phase C: out-projection, 2-stage so the first
                # half (head-0 dims, available after the first AllToAll)
                # overlaps the second head's attention ============
                # head-0 partials stay resident in SBUF — no DRAM bounce
                oc_sb = pb_res.tile([128, ET * 512], BF16, tag="ocs", bufs=1, name="ocs")
                with tc.tile_pool(name="ps_c", bufs=2, space="PSUM") as ps_c:
                    for ec in range(ET):
                        wo_sb = pb_stream.tile([128, 8 * 128], BF16, tag="wo", bufs=6, name=f"wo0_{ec}")
                        nc.sync.dma_start(
                            wo_sb[:], wout_p[:, ec * ET * 128 : ec * ET * 128 + 8 * 128]
                        )
                        ps = ps_c.tile([128, 512], F32, tag="u", bufs=2, name=f"psca{ec}")
                        for d in range(8):
                            nc.tensor.matmul(
                                ps[:],
                                wo_sb[:, d * 128 : (d + 1) * 128],
                                of_half[0][:, d * 512 : (d + 1) * 512],
                                start=(d == 0),
                                stop=(d == 7),
                            )
                        nc.scalar.copy(oc_sb[:, ec * 512 : (ec + 1) * 512], ps[:])
                    for ec in range(ET):
                        wo_sb = pb_stream.tile([128, 8 * 128], BF16, tag="wo", bufs=6, name=f"wo1_{ec}")
                        nc.sync.dma_start(
                            wo_sb[:], wout_p[:, ec * ET * 128 + 8 * 128 : (ec + 1) * ET * 128]
                        )
                        ps = ps_c.tile([128, 512], F32, tag="u", bufs=2, name=f"pscb{ec}")
                        for d in range(8):
                            nc.tensor.matmul(
                                ps[:],
                                wo_sb[:, d * 128 : (d + 1) * 128],
                                of_half[1][:, d * 512 : (d + 1) * 512],
                                start=(d == 0),
                                stop=(d == 7),
                            )
                        o_sb = pb_small.tile([128, 512], F32, tag="ocf", bufs=2, name=f"ocf{ec}")
                        nc.vector.tensor_tensor(
                            o_sb[:], ps[:], oc_sb[:, ec * 512 : (ec + 1) * 512],
                            op=mybir.AluOpType.add,
                        )
                        nc.sync.dma_start(out_t[ec * 128 : (ec + 1) * 128, :], o_sb[:])

    nc.compile()
    return nc


_NC_CACHE = None


def _get_program():
    global _NC_CACHE
    if _NC_CACHE is None:
        _NC_CACHE = build_program()
    return _NC_CACHE


def _host_tables():
    pos = np.arange(S, dtype=np.float32)
    inv_freq = 1.0 / (10000.0 ** (np.arange(0, RD, 2, dtype=np.float32) / RD))
    freqs = pos[:, None] * inv_freq[None, :]          # [S, 32]
    cos64 = np.concatenate([np.cos(freqs)] * 2, axis=1).T.astype(np.float32)  # [64, S]
    sin64 = np.sin(freqs).T.astype(np.float32)        # [32, S]
    sin_signed = np.concatenate([-sin64, sin64], axis=0)  # [64, S]
    cos_full = np.tile(cos64, (2, 1))                 # [128, S]
    sin_full = np.tile(sin_signed, (2, 1))            # [128, S]
    kk = np.arange(128)[:, None]
    qq = np.arange(512)[None, :]
    mask = np.concatenate(
        [(kk + o * 128 <= qq).astype(np.float32) for o in range(4)], axis=1
    ).astype(np.float16)                              # [128, 2048]
    return cos_full, sin_full, mask


def _pack_pm(w_t, n_in_tiles, n_out):
    """Pack [n_in_tiles*128, n_out] so chunk m is [128, n_in_tiles, 128] with
    long contiguous partition rows: out[p, ((m*n_in_tiles)+e)*128+f] = w_t[e*128+p, m*128+f]."""
    n_chunks = n_out // 128
    a = w_t.reshape(n_in_tiles, 128, n_chunks, 128).transpose(1, 2, 0, 3)
    return np.ascontiguousarray(a.reshape(128, n_chunks * n_in_tiles * 128))


def kernel(x, w_dq, w_uq, w_dkv, w_uk, w_uv, w_qr, w_kr, w_out):
    x = np.asarray(x, dtype=np.float32)
    w_dq = np.asarray(w_dq, dtype=np.float32)
    w_uq = np.asarray(w_uq, dtype=np.float32)
    w_dkv = np.asarray(w_dkv, dtype=np.float32)
    w_uk = np.asarray(w_uk, dtype=np.float32)
    w_uv = np.asarray(w_uv, dtype=np.float32)
    w_qr = np.asarray(w_qr, dtype=np.float32)
    w_kr = np.asarray(w_kr, dtype=np.float32)
    w_out = np.asarray(w_out, dtype=np.float32)

    nc = _get_program()
    cos_full, sin_full, mask = _host_tables()

    xt = np.ascontiguousarray(x.reshape(T, E).T)          # [E, T]
    wdq_p = _pack_pm(w_dq.T, ET, CQ).astype(ml_dtypes.bfloat16)
    wdkv_p = _pack_pm(w_dkv.T, ET, CKV).astype(ml_dtypes.bfloat16)
    wuq_p = _pack_pm(w_uq.T, CQT, H * HD).astype(ml_dtypes.bfloat16)
    wqr_p = _pack_pm(w_qr.T, CQT, H * RD).astype(ml_dtypes.bfloat16)
    wkr_p = _pack_pm(w_kr.T, ET, H * RD).astype(ml_dtypes.bfloat16)
    # permute w_out's input-dim tiles to [even heads, odd heads] to match the
    # head-split AllToAll reassembly in phase C
    perm = [2 * j for j in range(8)] + [2 * j + 1 for j in range(8)]
    wout_perm = w_out.T.reshape(ET, 128, E)[perm].reshape(E, E)
    wout_p = _pack_pm(wout_perm, ET, E).astype(ml_dtypes.bfloat16)
    ones_f16 = np.ones((128, 128), dtype=np.float16)

    in_maps = []
    for i in range(NC):
        hp = slice(i * HPC * HD, (i + 1) * HPC * HD)      # this core's head dims
        xt_loc = xt[:, i * TPC : (i + 1) * TPC]
        x_pi = np.ascontiguousarray(
            xt_loc.reshape(ET, 128, TPC).transpose(1, 0, 2).reshape(128, ET * TPC)
        ).astype(ml_dtypes.bfloat16)
        pos0 = (i % NBB) * 512
        in_maps.append(
            {
                "x_p": x_pi,
                "wdq_p": wdq_p,
                "wdkv_p": wdkv_p,
                "wkr_p": wkr_p,
                "wuq_p": wuq_p,
                "wqr_p": wqr_p,
                "wuk_p": _pack_pm(w_uk[hp, :].T, CKVT, HPC * HD).astype(ml_dtypes.bfloat16),
                # transposed pack for the direct-v^T matmuls:
                # wuv_p[p, c*256+hh] = w_uv[hp hh, c*128+p]
                "wuv_p": np.ascontiguousarray(
                    w_uv[hp, :].T.reshape(CKVT, 128, HPC * HD)
                    .transpose(1, 0, 2)
                    .reshape(128, CKVT * HPC * HD)
                ).astype(ml_dtypes.bfloat16),
                "wout_p": wout_p,
                "cos_t": np.ascontiguousarray(cos_full[:, pos0 : pos0 + 512]).astype(
                    ml_dtypes.bfloat16
                ),
                "sin_t": np.ascontiguousarray(sin_full[:, pos0 : pos0 + 512]).astype(
                    ml_dtypes.bfloat16
                ),
                "mask_t": mask,
                "ones_t": ones_f16,
            }
        )

    res = bass_utils.run_bass_kernel_spmd(nc, in_maps, core_ids=list(range(NC)))
    out = np.concatenate(
        [np.ascontiguousarray(res.results[i]["out_t"].T) for i in range(NC)], axis=0
    )
    return out.reshape(B, S, E)


def run_profiled(inputs):
    """Used by test.py: run once with NTFF tracing, return (output, exec_time_ns)."""
    sys.path.insert(0, "/root/.axon_site")
    from trn_agent_boot.trn_boot import _ntff_profile_via_ctypes

    hooks_mod = types.ModuleType("antenv.axon_hooks")
    hook = _ntff_profile_via_ctypes("/opt/axon/libaxon_pjrt.so")
    hooks_mod.get_axon_ntff_profile_hook = lambda: hook
    sys.modules["antenv.axon_hooks"] = hooks_mod

    orig = bass_utils.run_bass_kernel_spmd
    holder = {}

    def wrapper(nc, in_maps, core_ids, **kw):
        kw["trace"] = True
        res = orig(nc, in_maps, core_ids, **kw)
        holder["exec_time_ns"] = res.exec_time_ns
        return res

    bass_utils.run_bass_kernel_spmd = wrapper
    try:
        out = kernel(**inputs)
    finally:
        bass_utils.run_bass_kernel_spmd = orig
    return out, holder.get("exec_time_ns")
